# revision 2
# baseline (speedup 1.0000x reference)
"""Neural-ODE (Conv1d(76->76,k3)+tanh vector field, 8 fixed RK4 steps,
final Linear(76->15)) — Bass/Tile kernel for 8 TRN2 NeuronCores.

Sharding: data-parallel over the batch dim (64 -> 8 rows per core);
conv/linear weights replicated; the whole RK4 scan is device-local.

Per-core program (see emit_neural_ode): channels on partitions, time on
the free axis. Full integration stays in SBUF; x is loaded+transposed
(PE transpose) per group of RG rows, 8 RK4 steps run on-chip (each step
= 4 conv+tanh evals + fused-axpy glue), then the final linear layer is
applied via PE (bias folded in through a ones-row) and streamed out.

  h master state: fp32 [97, RG*TW] rows 0..75 = channels, row 96 = ones
  (bias trick), 4 zero guard cols around each row give conv zero-pad.
  k1..k4 / u2..u4 / t1,t2: bf16. Conv accumulation: PSUM fp32.
  tanh+bias fused on ScalarE reading PSUM directly.
  RK4 glue = fused scalar_tensor_tensor axpys, split DVE / GPSIMD.
"""

import sys

for _p in ("/opt/trn_rl_repo", "/root/.axon_site/_ro/trn_rl_repo"):
    if _p not in sys.path:
        sys.path.append(_p)

from contextlib import ExitStack

import numpy as np

import concourse.bass as bass
import concourse.mybir as mybir
import concourse.tile as tile
from concourse import bacc
from concourse.bass_utils import run_bass_kernel_spmd
from concourse.masks import make_identity

F32 = mybir.dt.float32
BF16 = mybir.dt.bfloat16
AF = mybir.ActivationFunctionType
ALU = mybir.AluOpType

N_CORES = 8
B_FULL = 64
T_FULL = 4096
F_CH = 76
C_CLS = 15
N_STEPS = 8
B_LOC = B_FULL // N_CORES


def emit_neural_ode(
    tc,
    out_ap,
    x_ap,
    conv_w_ap,
    conv_b_ap,
    final_w_ap,
    final_b_ap,
    n_steps: int = 8,
    rows_per_group: int = 2,
    dve_split: int = 2,
    lowp: bool = True,          # k/u in bf16 (False: all fp32, needs small T)
    glue_gpsimd: bool = True,   # offload u3/u4 axpys to GPSIMD
):
    nc = tc.nc
    B, T, F = x_ap.shape
    C = final_w_ap.shape[0]
    ONE = 96  # ones row partition (engine writes must start at 0/32/64/96)
    FP = ONE + 1
    G = 4
    TW = T + 2 * G
    dt = 1.0 / n_steps
    RG = rows_per_group
    assert B % RG == 0
    n_groups = B // RG
    PSW = 1024 if T % 1024 == 0 else 512
    assert T % PSW == 0
    NJ = T // PSW
    MMW = 512
    MH = PSW // MMW
    TCH = 128
    assert T % TCH == 0
    NT = T // TCH
    assert T % dve_split == 0
    HAL = T // dve_split

    KU_DT = BF16 if lowp else F32
    eng_dve = nc.vector
    eng_off = nc.gpsimd if glue_gpsimd else nc.vector

    ctx = ExitStack()
    with ctx:
        pers = ctx.enter_context(tc.tile_pool(name="pers", bufs=1))
        stage_pool = ctx.enter_context(tc.tile_pool(name="stage", bufs=4))
        zpool = ctx.enter_context(tc.tile_pool(name="zp", bufs=2, space="PSUM"))
        mpool = ctx.enter_context(tc.tile_pool(name="mp", bufs=2, space="PSUM"))

        # ---- persistent SBUF buffers ----
        h_buf = pers.tile([FP, RG * TW], F32, tag="h", name="h_buf")
        k_bufs = [
            [
                pers.tile([F, TW], KU_DT, tag=f"k{c}_{l}", name=f"k{c}_{l}")
                for l in range(4)
            ]
            for c in range(RG)
        ]
        u_bufs = [
            [
                pers.tile([F, TW], KU_DT, tag=f"u{c}_{i}", name=f"u{c}_{i}")
                for i in range(2)
            ]
            for c in range(RG)
        ]
        hs_bufs = [
            [
                pers.tile([F, T], KU_DT, tag=f"hs{c}_{i}", name=f"hs{c}_{i}")
                for i in range(2)
            ]
            for c in range(RG)
        ]
        w_f32 = pers.tile([F, 3 * F], F32, tag="wf", name="w_f32")
        bias_t = pers.tile([F, 1], F32, tag="bias", name="bias_t")
        fin_t = pers.tile([FP, C], F32, tag="fin", name="fin_t")
        ident = pers.tile([TCH, TCH], F32, tag="ident", name="ident")
        w_A = pers.tile([F, 3 * F], KU_DT, tag="wA", name="w_A")
        w_D = pers.tile([F, 3 * F], KU_DT, tag="wD", name="w_D")

        # ---- one-time init ----
        make_identity(nc, ident[:, :])
        nc.gpsimd.memset(fin_t[:, :], 0.0)
        nc.gpsimd.memset(h_buf[:, :], 0.0)
        nc.gpsimd.memset(h_buf[ONE:FP, :], 1.0)
        for c in range(RG):
            for i in range(2):
                nc.gpsimd.memset(u_bufs[c][i][:, :], 0.0)
        for d in range(3):
            nc.sync.dma_start(
                out=w_f32[0:F, d * F : (d + 1) * F],
                in_=conv_w_ap[:, :, d].rearrange("o i -> i o"),
            )
        nc.scalar.mul(w_A[:, :], w_f32[:, :], dt / 2.0)
        nc.scalar.mul(w_D[:, :], w_f32[:, :], dt)
        nc.sync.dma_start(
            out=bias_t[0:F, 0:1], in_=conv_b_ap.rearrange("(f o) -> f o", o=1)
        )
        nc.sync.dma_start(
            out=fin_t[0:F, 0:C], in_=final_w_ap.rearrange("c f -> f c")
        )
        nc.sync.dma_start(
            out=fin_t[ONE:FP, 0:C], in_=final_b_ap.rearrange("(o c) -> o c", o=1)
        )

        def hrow(c):
            # fp32 h slab offset for chain c (guarded row)
            return c * TW

        def emit_conv_level(c, rhs_buf, rhs_base, w_tile, k_out):
            """One conv+tanh eval over a full row for chain c."""
            for j in range(NJ):
                zt = zpool.tile([F, PSW], F32, tag="z", name="zt")
                for m in range(MH):
                    col0 = rhs_base + G + j * PSW + m * MMW - 1
                    for d in range(3):
                        nc.tensor.matmul(
                            zt[0:F, m * MMW : (m + 1) * MMW],
                            w_tile[0:F, d * F : (d + 1) * F],
                            rhs_buf[0:F, col0 + d : col0 + d + MMW],
                            start=(d == 0),
                            stop=(d == 2),
                        )
                nc.scalar.activation(
                    k_out[0:F, G + j * PSW : G + (j + 1) * PSW],
                    zt[0:F, 0:PSW],
                    AF.Tanh,
                    bias=bias_t[0:F, 0:1],
                )

        def emit_hsnap(c):
            # H = (2/dt)*h, H2 = (1/dt)*h   (bf16 snapshots, exact pow2 scale)
            H, H2 = hs_bufs[c]
            for hh in range(dve_split):
                sl = hh * HAL
                hsl = hrow(c) + G + hh * HAL
                eng_dve.tensor_scalar_mul(
                    H[0:F, sl : sl + HAL], h_buf[0:F, hsl : hsl + HAL], 2.0 / dt
                )
            for hh in range(dve_split):
                sl = hh * HAL
                eng_dve.tensor_scalar_mul(
                    H2[0:F, sl : sl + HAL], H[0:F, sl : sl + HAL], 0.5
                )

        def emit_uadd(eng, dst, hs, src_k):
            # dst[data] = hs + k   (plain add; scale lives in the weights)
            for hh in range(dve_split):
                sl = hh * HAL
                eng.tensor_add(
                    dst[0:F, G + sl : G + sl + HAL],
                    hs[0:F, sl : sl + HAL],
                    src_k[0:F, G + sl : G + sl + HAL],
                )

        def emit_kpair_accum(c, ka, kb, scal):
            # ka[data] += kb[data];  h += scal * ka   (fp32 accumulate)
            for hh in range(dve_split):
                sl = G + hh * HAL
                eng_dve.tensor_add(
                    ka[0:F, sl : sl + HAL],
                    ka[0:F, sl : sl + HAL],
                    kb[0:F, sl : sl + HAL],
                )
            for hh in range(dve_split):
                sl = G + hh * HAL
                hsl = hrow(c) + G + hh * HAL
                eng_dve.scalar_tensor_tensor(
                    out=h_buf[0:F, hsl : hsl + HAL],
                    in0=ka[0:F, sl : sl + HAL],
                    scalar=float(scal),
                    in1=h_buf[0:F, hsl : hsl + HAL],
                    op0=ALU.mult,
                    op1=ALU.add,
                )

        for g in range(n_groups):
            rows = [g * RG + c for c in range(RG)]

            # ---- load + transpose x rows into h ----
            for c, r in enumerate(rows):
                for j in range(NT):
                    st = stage_pool.tile([TCH, F], F32, tag="st", name="st")
                    nc.sync.dma_start(
                        out=st[:, :], in_=x_ap[r, j * TCH : (j + 1) * TCH, :]
                    )
                    pt = mpool.tile([F, TCH], F32, tag="xp", name="pt")
                    nc.tensor.transpose(pt[:, :], st[:, :], ident[:, :])
                    nc.vector.tensor_copy(
                        h_buf[0:F, hrow(c) + G + j * TCH : hrow(c) + G + (j + 1) * TCH],
                        pt[:, :],
                    )

            # ---- RK4 steps ----
            # u2 = h + dt/2 k1 = (dt/2)(H + k1): conv absorbs the scale.
            for s in range(n_steps):
                for c in range(RG):  # H/H2 snapshots of h (parallel with z1)
                    emit_hsnap(c)
                for c in range(RG):  # k1 = f(h)   [fp32 conv]
                    emit_conv_level(c, h_buf, hrow(c), w_f32, k_bufs[c][0])
                for c in range(RG):  # u2' = H + k1
                    emit_uadd(eng_dve, u_bufs[c][0], hs_bufs[c][0], k_bufs[c][0])
                for c in range(RG):  # k2 = f((dt/2) u2') via weights A
                    emit_conv_level(c, u_bufs[c][0], 0, w_A, k_bufs[c][1])
                for c in range(RG):  # u3' = H + k2
                    emit_uadd(eng_off, u_bufs[c][1], hs_bufs[c][0], k_bufs[c][1])
                for c in range(RG):  # k3 = f((dt/2) u3')
                    emit_conv_level(c, u_bufs[c][1], 0, w_A, k_bufs[c][2])
                for c in range(RG):  # u4' = H2 + k3 ; k2 += k3 ; h += dt/3 k2
                    emit_uadd(eng_off, u_bufs[c][0], hs_bufs[c][1], k_bufs[c][2])
                    emit_kpair_accum(c, k_bufs[c][1], k_bufs[c][2], dt / 3.0)
                for c in range(RG):  # k4 = f(dt u4') via weights D
                    emit_conv_level(c, u_bufs[c][0], 0, w_D, k_bufs[c][3])
                for c in range(RG):  # k1 += k4 ; h += dt/6 k1
                    emit_kpair_accum(c, k_bufs[c][0], k_bufs[c][3], dt / 6.0)

            # ---- final linear: out[t, c] = h[:, t]^T @ finT (+bias row) ----
            for c, r in enumerate(rows):
                for j in range(NT):
                    ps = mpool.tile([TCH, C], F32, tag="fo", name="ps")
                    nc.tensor.matmul(
                        ps[:, :],
                        h_buf[0:FP, hrow(c) + G + j * TCH : hrow(c) + G + (j + 1) * TCH],
                        fin_t[0:FP, 0:C],
                        start=True,
                        stop=True,
                    )
                    ob = stage_pool.tile([TCH, C], F32, tag="ob", name="ob")
                    nc.scalar.copy(ob[:, :], ps[:, :])
                    nc.sync.dma_start(
                        out=out_ap[r, j * TCH : (j + 1) * TCH, :], in_=ob[:, :]
                    )


_CACHE: dict = {}


def _build_program():
    if "nc" in _CACHE:
        return _CACHE["nc"]
    nc = bacc.Bacc(
        "TRN2",
        target_bir_lowering=False,
        debug=False,
        num_devices=N_CORES,
    )
    x = nc.dram_tensor("x", [B_LOC, T_FULL, F_CH], F32, kind="ExternalInput").ap()
    conv_w = nc.dram_tensor("conv_w", [F_CH, F_CH, 3], F32, kind="ExternalInput").ap()
    conv_b = nc.dram_tensor("conv_b", [F_CH], F32, kind="ExternalInput").ap()
    final_w = nc.dram_tensor("final_w", [C_CLS, F_CH], F32, kind="ExternalInput").ap()
    final_b = nc.dram_tensor("final_b", [C_CLS], F32, kind="ExternalInput").ap()
    out = nc.dram_tensor("out", [B_LOC, T_FULL, C_CLS], F32, kind="ExternalOutput").ap()

    with tile.TileContext(nc) as tc:
        emit_neural_ode(
            tc, out, x, conv_w, conv_b, final_w, final_b,
            n_steps=N_STEPS, rows_per_group=2, dve_split=2,
            lowp=True, glue_gpsimd=True,
        )
    nc.compile()
    _CACHE["nc"] = nc
    return nc


def _run(inputs: dict, trace: bool = False):
    nc = _build_program()
    x = np.ascontiguousarray(inputs["x"], dtype=np.float32)
    shared = {
        "conv_w": np.ascontiguousarray(inputs["conv_w"], dtype=np.float32),
        "conv_b": np.ascontiguousarray(inputs["conv_b"], dtype=np.float32),
        "final_w": np.ascontiguousarray(inputs["final_w"], dtype=np.float32),
        "final_b": np.ascontiguousarray(inputs["final_b"], dtype=np.float32),
    }
    in_maps = [
        {"x": np.ascontiguousarray(x[i * B_LOC : (i + 1) * B_LOC]), **shared}
        for i in range(N_CORES)
    ]
    res = run_bass_kernel_spmd(nc, in_maps, list(range(N_CORES)), trace=trace)
    out = np.concatenate([res.results[i]["out"] for i in range(N_CORES)], axis=0)
    return out, res


def kernel(x, conv_w, conv_b, final_w, final_b):
    out, _ = _run(
        {
            "x": x,
            "conv_w": conv_w,
            "conv_b": conv_b,
            "final_w": final_w,
            "final_b": final_b,
        }
    )
    return out
